# revision 34
# baseline (speedup 1.0000x reference)
"""Trainium2 Bass kernel for a dense transformer attention block.

Reference computation (fp32):
  q = rms_norm(x @ Wq.T)  per head (16 heads x 64)  -> rope -> * q_gain
  k = rms_norm(x @ Wk.T)  per kv-head (4 x 64)      -> rope
  v = x @ Wv.T
  causal GQA attention (16 q heads over 4 kv heads), softmax(q k / 8)
  out = (attn @ v) @ Wo.T

Sharding over 8 cores: core c = 2*b + hh handles batch b (of 4) and
q-head half hh (8 q heads = 2 kv heads).  Each core produces a partial
out [2048, 1024] (its heads' contribution through Wo); the host adds
the two partials per batch.  No collectives.

On-chip layout strategy:
  - host pre-transposes x and the weight slices, so all matmul operands
    arrive with the contraction dim on partitions (no input transposes)
  - scores are built transposed ([k, q]) so softmax needs no P transpose:
    exp(s) with the /8 scale and a -4 shift folded into the ACT op, the
    softmax denominator comes from an extra ones column in v, and the
    normalization is applied per-partition after the PV matmul
  - q_gain is folded into the host-built rope cos/sin tables
  - matmuls run as float32r (full-speed fp32); the attention-probability
    matmul runs in fp16 (safe: exp(s/8-4) <= e^8)
"""

import hashlib
import os

import numpy as np

# The libneuronxla NEFF cache can key-collide across different kernel
# versions with identical I/O shapes (observed: a stale NEFF served for an
# edited kernel).  Key the cache by this file's content so a changed kernel
# never hits a stale entry while identical re-runs stay warm.
try:
    _SRC_HASH = hashlib.sha256(open(__file__, "rb").read()).hexdigest()[:16]
except OSError:
    _SRC_HASH = "nosrc"
os.environ["NEURON_COMPILE_CACHE_URL"] = os.path.join(
    os.environ.get("TMPDIR", "/tmp"), f"neuron-cache-{_SRC_HASH}")

import concourse.bass as bass
import concourse.mybir as mybir
import concourse.tile as tile
from concourse import bacc
from concourse.bass_utils import run_bass_kernel_spmd
from concourse.masks import make_identity, make_upper_triangular

F32 = mybir.dt.float32
F32R = mybir.dt.float32r
F16 = mybir.dt.float16
AFT = mybir.ActivationFunctionType

B, S, D = 4, 2048, 1024
H, HD, KVH = 16, 64, 4
HL = 8            # q heads per core
KVL = 2           # kv heads per core
JQ = HL * HD      # 512 q-proj cols per core
JKV = KVL * HD    # 128 k (or v) proj cols per core
TT = S // 128     # 16 token tiles
DT = D // 128     # 8 contraction tiles
G = 4             # q groups of 512
ROPE_BASE = 10000.0
EPS = 1e-6
N_CORES = 8


def _build_program():
    nc = bacc.Bacc("TRN2", target_bir_lowering=False, debug=False,
                   num_devices=N_CORES)

    xT = nc.dram_tensor("xT", [D, S], F32R, kind="ExternalInput").ap()
    wqT = nc.dram_tensor("wqT", [D, JQ], F32R, kind="ExternalInput").ap()
    wkvT = nc.dram_tensor("wkvT", [D, 2 * JKV], F32R, kind="ExternalInput").ap()
    woT = nc.dram_tensor("woT", [JQ, D], F32R, kind="ExternalInput").ap()
    cosq = nc.dram_tensor("cosq", [S, HL * 32], F32, kind="ExternalInput").ap()
    sinq = nc.dram_tensor("sinq", [S, HL * 32], F32, kind="ExternalInput").ap()
    cosk = nc.dram_tensor("cosk", [S, KVL * 32], F32, kind="ExternalInput").ap()
    sink = nc.dram_tensor("sink", [S, KVL * 32], F32, kind="ExternalInput").ap()
    outp = nc.dram_tensor("outp", [S, D], F32, kind="ExternalOutput").ap()

    with tile.TileContext(nc) as tc:
        with (
            tc.tile_pool(name="consts", bufs=1) as consts,
            tc.tile_pool(name="persist", bufs=1) as persist,
        ):
            ident = consts.tile([128, 128], F32)
            make_identity(nc, ident)
            mask01 = consts.tile([128, 128], F16)
            make_upper_triangular(nc, mask01, val=1.0, diag=True)
            bias_eps = consts.tile([128, 1], F32)
            nc.gpsimd.memset(bias_eps[:], EPS)
            bias_m4 = consts.tile([128, 1], F32)
            nc.gpsimd.memset(bias_m4[:], -4.0)
            bias_0 = consts.tile([128, 1], F32)
            nc.gpsimd.memset(bias_0[:], 0.0)

            wq_sb = persist.tile([128, DT, JQ], F32R)
            wkv_sb = persist.tile([128, DT, 2 * JKV], F32R)
            wo_sb = persist.tile([128, JQ // 128, D], F32R)
            qT_sb = persist.tile([128, JQ // 128, S], F32R)
            kT_sb = persist.tile([128, S], F32R)
            v_sb = persist.tile([128, TT, 2 * (HD + 1)], F16)

            for dt in range(DT):
                nc.sync.dma_start(wq_sb[:, dt, :], wqT[128 * dt:128 * (dt + 1), :])
                nc.sync.dma_start(wkv_sb[:, dt, :], wkvT[128 * dt:128 * (dt + 1), :])
            for ft in range(JQ // 128):
                nc.sync.dma_start(wo_sb[:, ft, :], woT[128 * ft:128 * (ft + 1), :])

            # ones columns of v (softmax denominator comes out of the PV matmul)
            nc.gpsimd.memset(v_sb[:, :, HD:HD + 1], 1.0)
            nc.gpsimd.memset(v_sb[:, :, 2 * HD + 1:2 * HD + 2], 1.0)

            # ---------------- stage 1: projections, rms, rope, transposes
            with (
                tc.tile_pool(name="xpool", bufs=1) as xpool,
                tc.tile_pool(name="s1", bufs=3) as s1,
                tc.tile_pool(name="ps1", bufs=2, space="PSUM") as ps1,
            ):
                x_sb = xpool.tile([128, DT, S], F32R)
                for dt in range(DT):
                    nc.sync.dma_start(x_sb[:, dt, :], xT[128 * dt:128 * (dt + 1), :])

                for tt in range(TT):
                    tsl = slice(128 * tt, 128 * (tt + 1))

                    psq = ps1.tile([128, JQ], F32, tag="ps_q")
                    pskv = ps1.tile([128, 2 * JKV], F32, tag="ps_kv")
                    for dt in range(DT):
                        lhs = x_sb[:, dt, tsl]
                        nc.tensor.matmul(psq[:], lhs, wq_sb[:, dt, :],
                                         start=(dt == 0), stop=(dt == DT - 1))
                    for dt in range(DT):
                        lhs = x_sb[:, dt, tsl]
                        nc.tensor.matmul(pskv[:], lhs, wkv_sb[:, dt, :],
                                         start=(dt == 0), stop=(dt == DT - 1))

                    # rms statistics for q (8 heads) and k (2 heads).  DVE
                    # can't read PSUM twice in one op, so square against the
                    # SBUF evacuation copy.
                    q_sb = s1.tile([128, JQ], F32, tag="q_sb")
                    nc.scalar.copy(q_sb[:], psq[:])
                    k_sb = s1.tile([128, JKV], F32, tag="k_sb")
                    nc.scalar.copy(k_sb[:], pskv[:, 0:JKV])
                    sq = s1.tile([128, JQ], F32, tag="sq")
                    nc.vector.tensor_mul(sq[:], q_sb[:], psq[:])
                    sk = s1.tile([128, JKV], F32, tag="sk")
                    nc.vector.tensor_mul(sk[:], k_sb[:], pskv[:, 0:JKV])
                    st = s1.tile([128, HL + KVL, 1], F32, tag="st")
                    nc.vector.reduce_sum(out=st[:, 0:HL, :],
                                         in_=sq.rearrange("p (h f) -> p h f", h=HL),
                                         axis=mybir.AxisListType.X)
                    nc.vector.reduce_sum(out=st[:, HL:HL + KVL, :],
                                         in_=sk.rearrange("p (h f) -> p h f", h=KVL),
                                         axis=mybir.AxisListType.X)
                    # r = (mean_sq + eps) ** -0.5 on DVE (Newton; keeps ACT's
                    # table set pinned to exp).  Seed y0 = (1/m)*(a + b*m) has
                    # <= 13% rel err over m in [0.1, 2]; 3 iterations -> ~1e-6.
                    mm = s1.tile([128, HL + KVL, 1], F32, tag="mm")
                    nc.vector.tensor_scalar(mm[:], st[:], 1.0 / HD, EPS,
                                            mybir.AluOpType.mult,
                                            mybir.AluOpType.add)
                    st_w = s1.tile([128, HL + KVL, 1], F32, tag="st_w")
                    nc.vector.reciprocal(st_w[:], mm[:])
                    st_r = s1.tile([128, HL + KVL, 1], F32, tag="st_r")
                    nc.vector.tensor_scalar(st_r[:], mm[:], 0.657, 0.294,
                                            mybir.AluOpType.mult,
                                            mybir.AluOpType.add)
                    nc.vector.tensor_mul(st_r[:], st_r[:], st_w[:])
                    nt = s1.tile([128, HL + KVL, 1], F32, tag="nt")
                    for _ in range(3):
                        nc.vector.tensor_mul(nt[:], st_r[:], st_r[:])
                        nc.vector.tensor_mul(nt[:], nt[:], mm[:])
                        nc.vector.tensor_scalar(nt[:], nt[:], -0.5, 1.5,
                                                mybir.AluOpType.mult,
                                                mybir.AluOpType.add)
                        nc.vector.tensor_mul(st_r[:], st_r[:], nt[:])

                    # scale q/k by their rms factors (in place)
                    for h in range(HL):
                        nc.vector.tensor_scalar_mul(
                            q_sb[:, 64 * h:64 * (h + 1)],
                            q_sb[:, 64 * h:64 * (h + 1)], st_r[:, h, :])
                    for u in range(KVL):
                        nc.vector.tensor_scalar_mul(
                            k_sb[:, 64 * u:64 * (u + 1)],
                            k_sb[:, 64 * u:64 * (u + 1)], st_r[:, HL + u, :])
                    # v -> fp16 slots (per kv head: 64 values + the ones col)
                    for u in range(KVL):
                        nc.scalar.copy(
                            v_sb[:, tt, (HD + 1) * u:(HD + 1) * u + HD],
                            pskv[:, JKV + 64 * u:JKV + 64 * (u + 1)])

                    # rope tables for this token tile
                    cq = s1.tile([128, HL * 32], F32, tag="cq")
                    nc.sync.dma_start(cq[:], cosq[tsl, :])
                    sq_t = s1.tile([128, HL * 32], F32, tag="sq_t")
                    nc.sync.dma_start(sq_t[:], sinq[tsl, :])
                    ck = s1.tile([128, KVL * 32], F32, tag="ck")
                    nc.sync.dma_start(ck[:], cosk[tsl, :])
                    sk_t = s1.tile([128, KVL * 32], F32, tag="sk_t")
                    nc.sync.dma_start(sk_t[:], sink[tsl, :])

                    def rope(dst, src, cos_t, sin_t, nh, tmp):
                        s3 = src.rearrange("p (h f) -> p h f", h=nh)
                        d3 = dst.rearrange("p (h f) -> p h f", h=nh)
                        c3 = cos_t.rearrange("p (h f) -> p h f", h=nh)
                        n3 = sin_t.rearrange("p (h f) -> p h f", h=nh)
                        t1, t2 = s3[:, :, 0:32], s3[:, :, 32:64]
                        o1, o2 = d3[:, :, 0:32], d3[:, :, 32:64]
                        nc.vector.tensor_mul(o1, t1, c3)
                        nc.vector.tensor_mul(tmp[:], t2, n3)
                        nc.vector.tensor_add(o1, o1, tmp[:])
                        nc.vector.tensor_mul(o2, t2, c3)
                        nc.vector.tensor_mul(tmp[:], t1, n3)
                        nc.vector.tensor_sub(o2, o2, tmp[:])

                    qr = s1.tile([128, JQ], F32, tag="qr")
                    tmpq = s1.tile([128, HL, 32], F32, tag="tmpq")
                    rope(qr, q_sb, cq, sq_t, HL, tmpq)
                    kr = s1.tile([128, JKV], F32, tag="kr")
                    tmpk = s1.tile([128, KVL, 32], F32, tag="tmpk")
                    rope(kr, k_sb, ck, sk_t, KVL, tmpk)

                    # transpose q, k into feature-major layout.  q head h goes
                    # to qT_sb[64*(h//4) : +64, h%4, :] so its partition offset
                    # matches its kv head's offset in kT_sb (matmul requires
                    # equal base partitions for lhsT and rhs).
                    # q head SLOTS are host-permuted to [0,4,1,5,2,6,3,7] so a
                    # contiguous 128-col block holds the two heads that share
                    # a kv head at partition offsets {0, 64}.
                    for hp in range(4):
                        ptr = ps1.tile([128, 128], F32, tag="ps_tr")
                        nc.tensor.transpose(ptr[:], qr[:, 128 * hp:128 * (hp + 1)],
                                            ident[:])
                        nc.vector.tensor_copy(qT_sb[:, hp, tsl], ptr[:])
                    ptk = ps1.tile([128, 128], F32, tag="ps_tk", bufs=1)
                    nc.tensor.transpose(ptk[:], kr[:], ident[:])
                    nc.vector.tensor_copy(kT_sb[:, tsl], ptk[:])

            # ---------------- stage 2+3: attention + output projection
            with (
                tc.tile_pool(name="s2", bufs=2) as s2,
                tc.tile_pool(name="ps_s", bufs=2, space="PSUM") as psum_s,
                tc.tile_pool(name="ps_y", bufs=2, space="PSUM") as psum_y,
                tc.tile_pool(name="ps_t", bufs=1, space="PSUM") as psum_t,
                tc.tile_pool(name="ps_o", bufs=1, space="PSUM") as psum_o,
            ):
                for g in range(G):
                    y_sb = s2.tile([128, 4, JQ], F32, tag="y_sb")
                    for h in range(HL):
                        # h is the SLOT index; its kv head is u = h % 2 and it
                        # lives at qT_sb[64*(h%2) : +64, h//2, :]
                        u = h % 2
                        qrhs = qT_sb[64 * u:64 * (u + 1), h // 2,
                                     512 * g:512 * (g + 1)]
                        expT = s2.tile([128, 4 * g + 4, 512], F16, tag="expT")

                        # full rectangle k-tiles, 2 per PSUM batch (2 banks x
                        # bufs=2 keeps QK matmuls and exp evacuation pipelined)
                        for c in range(2 * g):
                            pss = psum_s.tile([128, 2, 512], F32, tag="pss")
                            for lane in range(2):
                                kt = 2 * c + lane
                                nc.tensor.matmul(
                                    pss[:, lane, :],
                                    kT_sb[64 * u:64 * (u + 1),
                                          128 * kt:128 * (kt + 1)],
                                    qrhs)
                            nc.scalar.activation(expT[:, 2 * c:2 * c + 2, :],
                                                 pss[:], AFT.Exp,
                                                 scale=0.125, bias=bias_m4[:])
                        # diagonal k-tiles (causal frontier)
                        for dc in range(2):
                            pss = psum_s.tile([128, 2, 512], F32, tag="pss")
                            for lane in range(2):
                                kt = 4 * g + 2 * dc + lane
                                n0 = 128 * (2 * dc + lane)
                                nc.tensor.matmul(
                                    pss[:, lane, n0:512],
                                    kT_sb[64 * u:64 * (u + 1),
                                          128 * kt:128 * (kt + 1)],
                                    qrhs[:, n0:512])
                                nc.scalar.activation(expT[:, kt, n0:512],
                                                     pss[:, lane, n0:512], AFT.Exp,
                                                     scale=0.125, bias=bias_m4[:])
                                nc.vector.tensor_mul(expT[:, kt, n0:n0 + 128],
                                                     expT[:, kt, n0:n0 + 128],
                                                     mask01[:])

                        # PV: y[q, f] (+ softmax denominator in the last col)
                        for i in range(4):
                            nkt = 4 * g + i + 1
                            psy = psum_y.tile([128, HD + 1], F32, tag="psy")
                            for kt in range(nkt):
                                nc.tensor.matmul(
                                    psy[:],
                                    expT[:, kt, 128 * i:128 * (i + 1)],
                                    v_sb[:, kt, (HD + 1) * u:(HD + 1) * (u + 1)],
                                    start=(kt == 0), stop=(kt == nkt - 1))
                            lc = s2.tile([128, 1], F32, tag="lc")
                            nc.vector.tensor_copy(lc[:], psy[:, HD:HD + 1])
                            rl = s2.tile([128, 1], F32, tag="rl")
                            nc.vector.reciprocal(rl[:], lc[:])
                            nc.vector.tensor_scalar_mul(
                                y_sb[:, i, 64 * h:64 * (h + 1)],
                                psy[:, 0:HD], rl[:])

                    # output projection for this group of 512 rows
                    for i in range(4):
                        yT = s2.tile([128, JQ // 128, 128], F32R, tag="yT")
                        for ft in range(JQ // 128):
                            ptt = psum_t.tile([128, 128], F32, tag="ptt")
                            nc.tensor.transpose(
                                ptt[:], y_sb[:, i, 128 * ft:128 * (ft + 1)],
                                ident[:])
                            nc.vector.tensor_copy(yT[:, ft, :], ptt[:])
                        out_sb = s2.tile([128, D], F32, tag="out_sb")
                        for nt in range(D // 512):
                            pso = psum_o.tile([128, 512], F32, tag="pso")
                            for ft in range(JQ // 128):
                                nc.tensor.matmul(
                                    pso[:], yT[:, ft, :],
                                    wo_sb[:, ft, 512 * nt:512 * (nt + 1)],
                                    start=(ft == 0), stop=(ft == JQ // 128 - 1))
                            nc.vector.tensor_copy(out_sb[:, 512 * nt:512 * (nt + 1)],
                                                  pso[:])
                        r0 = 512 * g + 128 * i
                        nc.sync.dma_start(outp[r0:r0 + 128, :], out_sb[:])

    nc.compile()
    return nc


_PROGRAM_CACHE = {}


def _rope_tables(gains):
    inv_freq = 1.0 / (ROPE_BASE ** (np.arange(0, HD, 2, dtype=np.float32) / HD))
    t = np.arange(S, dtype=np.float32)
    freqs = np.outer(t, inv_freq)                    # [S, 32]
    cos, sin = np.cos(freqs), np.sin(freqs)
    cos_g = np.concatenate([cos * g for g in gains], axis=1).astype(np.float32)
    sin_g = np.concatenate([sin * g for g in gains], axis=1).astype(np.float32)
    return np.ascontiguousarray(cos_g), np.ascontiguousarray(sin_g)


# q-head slot order: slot s holds local head PERM[s], so a contiguous
# 128-col block pairs the two heads sharing a kv head (see _build_program)
PERM = [0, 4, 1, 5, 2, 6, 3, 7]


def _in_map_for_core(x, Wq, Wk, Wv, Wo, q_gain, core):
    rows = np.concatenate([np.arange(64 * p, 64 * (p + 1)) for p in PERM])
    b, hh = core // 2, core % 2
    qh0 = JQ * hh
    kvh = slice(JKV * hh, JKV * (hh + 1))     # kv rows of Wk/Wv
    wkv = np.concatenate([Wk[kvh, :], Wv[kvh, :]], axis=0)  # [256, 1024]
    gains = q_gain[HL * hh:HL * (hh + 1)][PERM]
    cq, sq = _rope_tables(gains)
    ck, sk = _rope_tables(np.ones(KVL, dtype=np.float32))
    return {
        "xT": np.ascontiguousarray(x[b].T),
        "wqT": np.ascontiguousarray(Wq[qh0 + rows, :].T),
        "wkvT": np.ascontiguousarray(wkv.T),
        "woT": np.ascontiguousarray(Wo[:, qh0 + rows].T),
        "cosq": cq, "sinq": sq, "cosk": ck, "sink": sk,
    }


def kernel(x, Wq, Wk, Wv, Wo, q_gain):
    x = np.asarray(x, dtype=np.float32)
    Wq = np.asarray(Wq, dtype=np.float32)
    Wk = np.asarray(Wk, dtype=np.float32)
    Wv = np.asarray(Wv, dtype=np.float32)
    Wo = np.asarray(Wo, dtype=np.float32)
    q_gain = np.asarray(q_gain, dtype=np.float32)

    if "nc" not in _PROGRAM_CACHE:
        _PROGRAM_CACHE["nc"] = _build_program()
    nc = _PROGRAM_CACHE["nc"]

    in_maps = [_in_map_for_core(x, Wq, Wk, Wv, Wo, q_gain, core)
               for core in range(N_CORES)]

    res = run_bass_kernel_spmd(nc, in_maps, core_ids=list(range(N_CORES)))
    _PROGRAM_CACHE["last_results"] = res

    out = np.empty((B, S, D), dtype=np.float32)
    for b in range(B):
        out[b] = res.results[2 * b]["outp"] + res.results[2 * b + 1]["outp"]
    return out


if __name__ == "__main__":
    rng = np.random.default_rng(0)
    inputs = {
        "x": rng.standard_normal((B, S, D), dtype=np.float32),
        "Wq": rng.standard_normal((D, D), dtype=np.float32) * 0.02,
        "Wk": rng.standard_normal((KVH * HD, D), dtype=np.float32) * 0.02,
        "Wv": rng.standard_normal((KVH * HD, D), dtype=np.float32) * 0.02,
        "Wo": rng.standard_normal((D, D), dtype=np.float32) * 0.02,
        "q_gain": np.full((H,), 1.5, dtype=np.float32),
    }
    out = kernel(**inputs)
    print(out.shape, out.dtype, np.abs(out).max())


# revision 39
# speedup vs baseline: 1.0154x; 1.0154x over previous
"""Trainium2 Bass kernel for a dense transformer attention block.

Reference computation (fp32):
  q = rms_norm(x @ Wq.T)  per head (16 heads x 64)  -> rope -> * q_gain
  k = rms_norm(x @ Wk.T)  per kv-head (4 x 64)      -> rope
  v = x @ Wv.T
  causal GQA attention (16 q heads over 4 kv heads), softmax(q k / 8)
  out = (attn @ v) @ Wo.T

Sharding over 8 cores: core c = 2*b + hh handles batch b (of 4) and
q-head half hh (8 q heads = 2 kv heads).  Each core produces a partial
out [2048, 1024] (its heads' contribution through Wo); the host adds
the two partials per batch.  No collectives.

On-chip layout strategy:
  - host pre-transposes x and the weight slices, so all matmul operands
    arrive with the contraction dim on partitions (no input transposes)
  - scores are built transposed ([k, q]) so softmax needs no P transpose:
    exp(s) with the /8 scale and a -4 shift folded into the ACT op, the
    softmax denominator comes from an extra ones column in v, and the
    normalization is applied per-partition after the PV matmul
  - q_gain is folded into the host-built rope cos/sin tables
  - matmuls run as float32r (full-speed fp32); the attention-probability
    matmul runs in fp16 (safe: exp(s/8-4) <= e^8)
"""

import hashlib
import os

import numpy as np

# The libneuronxla NEFF cache can key-collide across different kernel
# versions with identical I/O shapes (observed: a stale NEFF served for an
# edited kernel).  Key the cache by this file's content so a changed kernel
# never hits a stale entry while identical re-runs stay warm.
try:
    _SRC_HASH = hashlib.sha256(open(__file__, "rb").read()).hexdigest()[:16]
except OSError:
    _SRC_HASH = "nosrc"
os.environ["NEURON_COMPILE_CACHE_URL"] = os.path.join(
    os.environ.get("TMPDIR", "/tmp"), f"neuron-cache-{_SRC_HASH}")

import concourse.bass as bass
import concourse.mybir as mybir
import concourse.tile as tile
from concourse import bacc
from concourse.bass_utils import run_bass_kernel_spmd
from concourse.masks import make_identity, make_upper_triangular

F32 = mybir.dt.float32
F32R = mybir.dt.float32r
F16 = mybir.dt.float16
AFT = mybir.ActivationFunctionType

B, S, D = 4, 2048, 1024
H, HD, KVH = 16, 64, 4
HL = 8            # q heads per core
KVL = 2           # kv heads per core
JQ = HL * HD      # 512 q-proj cols per core
JKV = KVL * HD    # 128 k (or v) proj cols per core
TT = S // 128     # 16 token tiles
DT = D // 128     # 8 contraction tiles
G = 4             # q groups of 512
ROPE_BASE = 10000.0
EPS = 1e-6
N_CORES = 8


def _build_program():
    nc = bacc.Bacc("TRN2", target_bir_lowering=False, debug=False,
                   num_devices=N_CORES)

    xT = nc.dram_tensor("xT", [D, S], F32R, kind="ExternalInput").ap()
    wqT = nc.dram_tensor("wqT", [D, JQ], F32R, kind="ExternalInput").ap()
    wkvT = nc.dram_tensor("wkvT", [D, 2 * JKV], F32R, kind="ExternalInput").ap()
    woT = nc.dram_tensor("woT", [JQ, D], F32R, kind="ExternalInput").ap()
    cosq = nc.dram_tensor("cosq", [S, HL * 32], F32, kind="ExternalInput").ap()
    sinq = nc.dram_tensor("sinq", [S, HL * 32], F32, kind="ExternalInput").ap()
    cosk = nc.dram_tensor("cosk", [S, KVL * 32], F32, kind="ExternalInput").ap()
    sink = nc.dram_tensor("sink", [S, KVL * 32], F32, kind="ExternalInput").ap()
    outp = nc.dram_tensor("outp", [S, D], F32, kind="ExternalOutput").ap()

    with tile.TileContext(nc) as tc:
        with (
            tc.tile_pool(name="consts", bufs=1) as consts,
            tc.tile_pool(name="persist", bufs=1) as persist,
        ):
            ident = consts.tile([128, 128], F32)
            make_identity(nc, ident)
            mask01 = consts.tile([128, 128], F16)
            make_upper_triangular(nc, mask01, val=1.0, diag=True)
            bias_eps = consts.tile([128, 1], F32)
            nc.gpsimd.memset(bias_eps[:], EPS)
            bias_m4 = consts.tile([128, 1], F32)
            nc.gpsimd.memset(bias_m4[:], -4.0)
            bias_0 = consts.tile([128, 1], F32)
            nc.gpsimd.memset(bias_0[:], 0.0)

            wq_sb = persist.tile([128, DT, JQ], F32R)
            wkv_sb = persist.tile([128, DT, 2 * JKV], F32R)
            wo_sb = persist.tile([128, JQ // 128, D], F32R)
            qT_sb = persist.tile([128, JQ // 128, S], F32R)
            kT_sb = persist.tile([128, S], F32R)
            v_sb = persist.tile([128, TT, 2 * (HD + 1)], F16)

            for dt in range(DT):
                nc.sync.dma_start(wkv_sb[:, dt, :], wkvT[128 * dt:128 * (dt + 1), :])
                nc.sync.dma_start(wq_sb[:, dt, :], wqT[128 * dt:128 * (dt + 1), :])

            # ones columns of v (softmax denominator comes out of the PV matmul)
            nc.gpsimd.memset(v_sb[:, :, HD:HD + 1], 1.0)
            nc.gpsimd.memset(v_sb[:, :, 2 * HD + 1:2 * HD + 2], 1.0)

            # ---------------- stage 1: projections, rms, rope, transposes
            with (
                tc.tile_pool(name="xpool", bufs=1) as xpool,
                tc.tile_pool(name="s1", bufs=3) as s1,
                tc.tile_pool(name="ps1", bufs=2, space="PSUM") as ps1,
            ):
                x_sb = xpool.tile([128, DT, S], F32R)
                for dt in range(DT):
                    nc.sync.dma_start(x_sb[:, dt, :], xT[128 * dt:128 * (dt + 1), :])

                for tt in range(TT):
                    tsl = slice(128 * tt, 128 * (tt + 1))

                    psq = ps1.tile([128, JQ], F32, tag="ps_q")
                    pskv = ps1.tile([128, 2 * JKV], F32, tag="ps_kv")
                    for dt in range(DT):
                        lhs = x_sb[:, dt, tsl]
                        nc.tensor.matmul(psq[:], lhs, wq_sb[:, dt, :],
                                         start=(dt == 0), stop=(dt == DT - 1))
                    for dt in range(DT):
                        lhs = x_sb[:, dt, tsl]
                        nc.tensor.matmul(pskv[:], lhs, wkv_sb[:, dt, :],
                                         start=(dt == 0), stop=(dt == DT - 1))

                    # rms statistics for q (8 heads) and k (2 heads).  DVE
                    # can't read PSUM twice in one op, so square against the
                    # SBUF evacuation copy.
                    q_sb = s1.tile([128, JQ], F32, tag="q_sb")
                    nc.scalar.copy(q_sb[:], psq[:])
                    k_sb = s1.tile([128, JKV], F32, tag="k_sb")
                    nc.scalar.copy(k_sb[:], pskv[:, 0:JKV])
                    sq = s1.tile([128, JQ], F32, tag="sq")
                    nc.vector.tensor_mul(sq[:], q_sb[:], psq[:])
                    sk = s1.tile([128, JKV], F32, tag="sk")
                    nc.vector.tensor_mul(sk[:], k_sb[:], pskv[:, 0:JKV])
                    st = s1.tile([128, HL + KVL, 1], F32, tag="st")
                    nc.vector.reduce_sum(out=st[:, 0:HL, :],
                                         in_=sq.rearrange("p (h f) -> p h f", h=HL),
                                         axis=mybir.AxisListType.X)
                    nc.vector.reduce_sum(out=st[:, HL:HL + KVL, :],
                                         in_=sk.rearrange("p (h f) -> p h f", h=KVL),
                                         axis=mybir.AxisListType.X)
                    # r = (mean_sq + eps) ** -0.5 on DVE (Newton; keeps ACT's
                    # table set pinned to exp).  Seed y0 = (1/m)*(a + b*m) has
                    # <= 13% rel err over m in [0.1, 2]; 3 iterations -> ~1e-6.
                    mm = s1.tile([128, HL + KVL, 1], F32, tag="mm")
                    nc.vector.tensor_scalar(mm[:], st[:], 1.0 / HD, EPS,
                                            mybir.AluOpType.mult,
                                            mybir.AluOpType.add)
                    st_w = s1.tile([128, HL + KVL, 1], F32, tag="st_w")
                    nc.vector.reciprocal(st_w[:], mm[:])
                    st_r = s1.tile([128, HL + KVL, 1], F32, tag="st_r")
                    nc.vector.tensor_scalar(st_r[:], mm[:], 0.657, 0.294,
                                            mybir.AluOpType.mult,
                                            mybir.AluOpType.add)
                    nc.vector.tensor_mul(st_r[:], st_r[:], st_w[:])
                    nt = s1.tile([128, HL + KVL, 1], F32, tag="nt")
                    for _ in range(3):
                        nc.vector.tensor_mul(nt[:], st_r[:], st_r[:])
                        nc.vector.tensor_mul(nt[:], nt[:], mm[:])
                        nc.vector.tensor_scalar(nt[:], nt[:], -0.5, 1.5,
                                                mybir.AluOpType.mult,
                                                mybir.AluOpType.add)
                        nc.vector.tensor_mul(st_r[:], st_r[:], nt[:])

                    # scale q/k by their rms factors (in place)
                    for h in range(HL):
                        nc.vector.tensor_scalar_mul(
                            q_sb[:, 64 * h:64 * (h + 1)],
                            q_sb[:, 64 * h:64 * (h + 1)], st_r[:, h, :])
                    for u in range(KVL):
                        nc.vector.tensor_scalar_mul(
                            k_sb[:, 64 * u:64 * (u + 1)],
                            k_sb[:, 64 * u:64 * (u + 1)], st_r[:, HL + u, :])
                    # v -> fp16 slots (per kv head: 64 values + the ones col)
                    for u in range(KVL):
                        nc.scalar.copy(
                            v_sb[:, tt, (HD + 1) * u:(HD + 1) * u + HD],
                            pskv[:, JKV + 64 * u:JKV + 64 * (u + 1)])

                    # rope tables for this token tile
                    cq = s1.tile([128, HL * 32], F32, tag="cq")
                    nc.sync.dma_start(cq[:], cosq[tsl, :])
                    sq_t = s1.tile([128, HL * 32], F32, tag="sq_t")
                    nc.sync.dma_start(sq_t[:], sinq[tsl, :])
                    ck = s1.tile([128, KVL * 32], F32, tag="ck")
                    nc.sync.dma_start(ck[:], cosk[tsl, :])
                    sk_t = s1.tile([128, KVL * 32], F32, tag="sk_t")
                    nc.sync.dma_start(sk_t[:], sink[tsl, :])

                    def rope(dst, src, cos_t, sin_t, nh, tmp):
                        s3 = src.rearrange("p (h f) -> p h f", h=nh)
                        d3 = dst.rearrange("p (h f) -> p h f", h=nh)
                        c3 = cos_t.rearrange("p (h f) -> p h f", h=nh)
                        n3 = sin_t.rearrange("p (h f) -> p h f", h=nh)
                        t1, t2 = s3[:, :, 0:32], s3[:, :, 32:64]
                        o1, o2 = d3[:, :, 0:32], d3[:, :, 32:64]
                        nc.vector.tensor_mul(o1, t1, c3)
                        nc.vector.tensor_mul(tmp[:], t2, n3)
                        nc.vector.tensor_add(o1, o1, tmp[:])
                        nc.vector.tensor_mul(o2, t2, c3)
                        nc.vector.tensor_mul(tmp[:], t1, n3)
                        nc.vector.tensor_sub(o2, o2, tmp[:])

                    qr = s1.tile([128, JQ], F32, tag="qr")
                    tmpq = s1.tile([128, HL, 32], F32, tag="tmpq")
                    rope(qr, q_sb, cq, sq_t, HL, tmpq)
                    kr = s1.tile([128, JKV], F32, tag="kr")
                    tmpk = s1.tile([128, KVL, 32], F32, tag="tmpk")
                    rope(kr, k_sb, ck, sk_t, KVL, tmpk)

                    # transpose q, k into feature-major layout.  q head h goes
                    # to qT_sb[64*(h//4) : +64, h%4, :] so its partition offset
                    # matches its kv head's offset in kT_sb (matmul requires
                    # equal base partitions for lhsT and rhs).
                    # q head SLOTS are host-permuted to [0,4,1,5,2,6,3,7] so a
                    # contiguous 128-col block holds the two heads that share
                    # a kv head at partition offsets {0, 64}.
                    for hp in range(4):
                        ptr = ps1.tile([128, 128], F32, tag="ps_tr")
                        nc.tensor.transpose(ptr[:], qr[:, 128 * hp:128 * (hp + 1)],
                                            ident[:])
                        nc.vector.tensor_copy(qT_sb[:, hp, tsl], ptr[:])
                    ptk = ps1.tile([128, 128], F32, tag="ps_tk", bufs=1)
                    nc.tensor.transpose(ptk[:], kr[:], ident[:])
                    nc.vector.tensor_copy(kT_sb[:, tsl], ptk[:])

            # ---------------- stage 2+3: attention + output projection
            for ft in range(JQ // 128):
                nc.sync.dma_start(wo_sb[:, ft, :], woT[128 * ft:128 * (ft + 1), :])
            with (
                tc.tile_pool(name="s2", bufs=2) as s2,
                tc.tile_pool(name="ps_s", bufs=2, space="PSUM") as psum_s,
                tc.tile_pool(name="ps_y", bufs=2, space="PSUM") as psum_y,
                tc.tile_pool(name="ps_t", bufs=1, space="PSUM") as psum_t,
                tc.tile_pool(name="ps_o", bufs=1, space="PSUM") as psum_o,
            ):
                for g in range(G):
                    y_sb = s2.tile([128, 4, JQ], F32, tag="y_sb")
                    for h in range(HL):
                        # h is the SLOT index; its kv head is u = h % 2 and it
                        # lives at qT_sb[64*(h%2) : +64, h//2, :]
                        u = h % 2
                        qrhs = qT_sb[64 * u:64 * (u + 1), h // 2,
                                     512 * g:512 * (g + 1)]
                        expT = s2.tile([128, 4 * g + 4, 512], F16, tag="expT")

                        # full rectangle k-tiles, 2 per PSUM batch (2 banks x
                        # bufs=2 keeps QK matmuls and exp evacuation pipelined)
                        for c in range(2 * g):
                            pss = psum_s.tile([128, 2, 512], F32, tag="pss")
                            for lane in range(2):
                                kt = 2 * c + lane
                                nc.tensor.matmul(
                                    pss[:, lane, :],
                                    kT_sb[64 * u:64 * (u + 1),
                                          128 * kt:128 * (kt + 1)],
                                    qrhs)
                            nc.scalar.activation(expT[:, 2 * c:2 * c + 2, :],
                                                 pss[:], AFT.Exp,
                                                 scale=0.125, bias=bias_m4[:])
                        # diagonal k-tiles (causal frontier)
                        for dc in range(2):
                            pss = psum_s.tile([128, 2, 512], F32, tag="pss")
                            for lane in range(2):
                                kt = 4 * g + 2 * dc + lane
                                n0 = 128 * (2 * dc + lane)
                                nc.tensor.matmul(
                                    pss[:, lane, n0:512],
                                    kT_sb[64 * u:64 * (u + 1),
                                          128 * kt:128 * (kt + 1)],
                                    qrhs[:, n0:512])
                                nc.scalar.activation(expT[:, kt, n0:512],
                                                     pss[:, lane, n0:512], AFT.Exp,
                                                     scale=0.125, bias=bias_m4[:])
                                nc.vector.tensor_mul(expT[:, kt, n0:n0 + 128],
                                                     expT[:, kt, n0:n0 + 128],
                                                     mask01[:])

                        # PV: y[q, f] (+ softmax denominator in the last col)
                        for i in range(4):
                            nkt = 4 * g + i + 1
                            psy = psum_y.tile([128, HD + 1], F32, tag="psy")
                            for kt in range(nkt):
                                nc.tensor.matmul(
                                    psy[:],
                                    expT[:, kt, 128 * i:128 * (i + 1)],
                                    v_sb[:, kt, (HD + 1) * u:(HD + 1) * (u + 1)],
                                    start=(kt == 0), stop=(kt == nkt - 1))
                            lc = s2.tile([128, 1], F32, tag="lc")
                            nc.vector.tensor_copy(lc[:], psy[:, HD:HD + 1])
                            rl = s2.tile([128, 1], F32, tag="rl")
                            nc.vector.reciprocal(rl[:], lc[:])
                            nc.vector.tensor_scalar_mul(
                                y_sb[:, i, 64 * h:64 * (h + 1)],
                                psy[:, 0:HD], rl[:])

                    # output projection for this group of 512 rows
                    for i in range(4):
                        yT = s2.tile([128, JQ // 128, 128], F32R, tag="yT")
                        for ft in range(JQ // 128):
                            ptt = psum_t.tile([128, 128], F32, tag="ptt")
                            nc.tensor.transpose(
                                ptt[:], y_sb[:, i, 128 * ft:128 * (ft + 1)],
                                ident[:])
                            nc.vector.tensor_copy(yT[:, ft, :], ptt[:])
                        out_sb = s2.tile([128, D], F32, tag="out_sb")
                        for nt in range(D // 512):
                            pso = psum_o.tile([128, 512], F32, tag="pso")
                            for ft in range(JQ // 128):
                                nc.tensor.matmul(
                                    pso[:], yT[:, ft, :],
                                    wo_sb[:, ft, 512 * nt:512 * (nt + 1)],
                                    start=(ft == 0), stop=(ft == JQ // 128 - 1))
                            nc.vector.tensor_copy(out_sb[:, 512 * nt:512 * (nt + 1)],
                                                  pso[:])
                        r0 = 512 * g + 128 * i
                        nc.sync.dma_start(outp[r0:r0 + 128, :], out_sb[:])

    nc.compile()
    return nc


_PROGRAM_CACHE = {}


def _rope_tables(gains):
    inv_freq = 1.0 / (ROPE_BASE ** (np.arange(0, HD, 2, dtype=np.float32) / HD))
    t = np.arange(S, dtype=np.float32)
    freqs = np.outer(t, inv_freq)                    # [S, 32]
    cos, sin = np.cos(freqs), np.sin(freqs)
    cos_g = np.concatenate([cos * g for g in gains], axis=1).astype(np.float32)
    sin_g = np.concatenate([sin * g for g in gains], axis=1).astype(np.float32)
    return np.ascontiguousarray(cos_g), np.ascontiguousarray(sin_g)


# q-head slot order: slot s holds local head PERM[s], so a contiguous
# 128-col block pairs the two heads sharing a kv head (see _build_program)
PERM = [0, 4, 1, 5, 2, 6, 3, 7]


def _in_map_for_core(x, Wq, Wk, Wv, Wo, q_gain, core):
    rows = np.concatenate([np.arange(64 * p, 64 * (p + 1)) for p in PERM])
    b, hh = core // 2, core % 2
    qh0 = JQ * hh
    kvh = slice(JKV * hh, JKV * (hh + 1))     # kv rows of Wk/Wv
    wkv = np.concatenate([Wk[kvh, :], Wv[kvh, :]], axis=0)  # [256, 1024]
    gains = q_gain[HL * hh:HL * (hh + 1)][PERM]
    cq, sq = _rope_tables(gains)
    ck, sk = _rope_tables(np.ones(KVL, dtype=np.float32))
    return {
        "xT": np.ascontiguousarray(x[b].T),
        "wqT": np.ascontiguousarray(Wq[qh0 + rows, :].T),
        "wkvT": np.ascontiguousarray(wkv.T),
        "woT": np.ascontiguousarray(Wo[:, qh0 + rows].T),
        "cosq": cq, "sinq": sq, "cosk": ck, "sink": sk,
    }


def kernel(x, Wq, Wk, Wv, Wo, q_gain):
    x = np.asarray(x, dtype=np.float32)
    Wq = np.asarray(Wq, dtype=np.float32)
    Wk = np.asarray(Wk, dtype=np.float32)
    Wv = np.asarray(Wv, dtype=np.float32)
    Wo = np.asarray(Wo, dtype=np.float32)
    q_gain = np.asarray(q_gain, dtype=np.float32)

    if "nc" not in _PROGRAM_CACHE:
        _PROGRAM_CACHE["nc"] = _build_program()
    nc = _PROGRAM_CACHE["nc"]

    in_maps = [_in_map_for_core(x, Wq, Wk, Wv, Wo, q_gain, core)
               for core in range(N_CORES)]

    res = run_bass_kernel_spmd(nc, in_maps, core_ids=list(range(N_CORES)))
    _PROGRAM_CACHE["last_results"] = res

    out = np.empty((B, S, D), dtype=np.float32)
    for b in range(B):
        out[b] = res.results[2 * b]["outp"] + res.results[2 * b + 1]["outp"]
    return out


if __name__ == "__main__":
    rng = np.random.default_rng(0)
    inputs = {
        "x": rng.standard_normal((B, S, D), dtype=np.float32),
        "Wq": rng.standard_normal((D, D), dtype=np.float32) * 0.02,
        "Wk": rng.standard_normal((KVH * HD, D), dtype=np.float32) * 0.02,
        "Wv": rng.standard_normal((KVH * HD, D), dtype=np.float32) * 0.02,
        "Wo": rng.standard_normal((D, D), dtype=np.float32) * 0.02,
        "q_gain": np.full((H,), 1.5, dtype=np.float32),
    }
    out = kernel(**inputs)
    print(out.shape, out.dtype, np.abs(out).max())
